# revision 7
# baseline (speedup 1.0000x reference)
"""DSSM (vision Mamba block) Trainium2 kernel — v4.

Problem: B=4, H=W=48, L=2304, D_MODEL=96, D_INNER=192, N=16, R=6, K=3.

Sharding (8 cores, no device-to-device comms):
  core c -> batch b=c//2, d_inner half=c%2. Each core runs the full-d
  front-end (in_proj, depthwise conv, x_dbl) for its batch, the K=3
  selective scans for its 96 d_inner channels, and a partial out_proj
  (contraction over its d-half). Host sums the two partials per batch.

Design notes (measured on HW):
  - DVE scan = ~2.0 ns/col and is the irreducible bottleneck
    (36 groups x [128,2304]); everything else hides under it.
  - x is transposed to [96, L] bf16 on the HOST (the row-major on-device
    load cost ~115us of per-row DMA descriptors).
  - Delta / Delta*u are partition-replicated [96 -> 128=(d8,n16)] by PE
    0/1 matmuls; ACT consumes PSUM directly: a = exp(A*Delta) with a
    per-partition scale vector (exact A), dub copied to SBUF bf16 so the
    w-mul keeps the 2x bf16 DVE rate.
  - all matmuls bf16 (fp32 emits 2 half-speed instructions).
  - LDWEIGHTS elided (inst.ldweights=False) wherever consecutive matmuls
    share a stationary tensor; loops are ordered weight-outer for this.
  - front-end shares the 5 psY PSUM banks as its pipeline ring; psR
    (bufs=3) serves the per-direction streams. 5 + 3 = 8 banks.
  - GpSimd is avoided for bulk work: it contends with DVE SBUF ports
    (+75% measured on concurrent DVE ops).
"""

import numpy as np
import ml_dtypes

import concourse.bass as bass
import concourse.mybir as mybir
import concourse.tile as tile
from concourse.bass_utils import run_bass_kernel_spmd

# ---------------------------------------------------------------- tile fix
# The walrus here accepts only ONE inline sem-wait per instruction; Tile can
# attach several. Hoist extras onto same-engine NOPs placed just before.
_wsplit_counter = [0]


def _split_multi_waits(nc):
    for fn in nc.m.functions:
        for blk in fn.blocks:
            out = []
            changed = False
            for inst in blk.instructions:
                si = inst.sync_info
                waits = list(si.on_wait) if si is not None and si.on_wait else []
                if len(waits) > 1:
                    changed = True
                    for w in waits[:-1]:
                        _wsplit_counter[0] += 1
                        nop = mybir.InstNoOp(name=f"wsplit-{_wsplit_counter[0]}")
                        nop.engine = inst.engine
                        nop.sync_info = mybir.SyncInfo(on_wait=[w], on_update=[])
                        out.append(nop)
                    inst.sync_info = mybir.SyncInfo(
                        on_wait=[waits[-1]],
                        on_update=list(si.on_update) if si.on_update else [],
                    )
                out.append(inst)
            if changed:
                blk.instructions = out


class TileContextFixed(tile.TileContext):
    def __exit__(self, exc_type, exc_val, exc_tb):
        r = super().__exit__(exc_type, exc_val, exc_tb)
        if exc_type is None:
            _split_multi_waits(self.nc)
        return r


# ---------------------------------------------------------------- constants
B, H, W = 4, 48, 48
DM, DI, N, R, K = 96, 192, 16, 6, 3
L = H * W
DH = 96          # d-half per core
G = DH // 8      # 12 groups of 8 channels
TILES = [(0, 480), (480, 960), (960, 1440), (1440, 1920), (1920, 2304)]

F32 = mybir.dt.float32
BF16 = mybir.dt.bfloat16
MUL = mybir.AluOpType.mult
ADD = mybir.AluOpType.add
AF = mybir.ActivationFunctionType

_COMPILED = {}


def _build_nc():
    nc = bass.Bass()

    # ---- dram I/O (per-core values supplied via in_maps)
    xt_in = nc.dram_tensor("xt_in", [DM, L], BF16, kind="ExternalInput")
    wxz = nc.dram_tensor("wxz", [DM, 288], BF16, kind="ExternalInput")
    conv_diag = nc.dram_tensor("conv_diag", [DH, 18 * DH], BF16, kind="ExternalInput")
    conv_bias = nc.dram_tensor("conv_bias", [DH, 2], F32, kind="ExternalInput")
    xp_T = nc.dram_tensor("xp_T", [DH, K * 2 * 80], BF16, kind="ExternalInput")
    dtw_T = nc.dram_tensor("dtw_T", [R, K * DH], BF16, kind="ExternalInput")
    dt_bias = nc.dram_tensor("dt_bias", [DH, K], F32, kind="ExternalInput")
    scales = nc.dram_tensor("scales", [128, K * G], F32, kind="ExternalInput")
    wi8 = nc.dram_tensor("wi8", [DH, G * 128], BF16, kind="ExternalInput")
    wbc = nc.dram_tensor("wbc", [80, 128], BF16, kind="ExternalInput")
    wr12 = nc.dram_tensor("wr12", [128, G * DH], BF16, kind="ExternalInput")
    ds_diag = nc.dram_tensor("ds_diag", [DH, K * DH], BF16, kind="ExternalInput")
    wout_T = nc.dram_tensor("wout_T", [DH, DM], BF16, kind="ExternalInput")
    out_part = nc.dram_tensor("out_part", [DM, L], F32, kind="ExternalOutput")

    with TileContextFixed(nc) as tc:
        with (
            tc.tile_pool(name="wts", bufs=1) as wts,
            tc.tile_pool(name="big", bufs=1) as big,
            tc.tile_pool(name="perd", bufs=2) as perd,
            tc.tile_pool(name="grp", bufs=2) as grp,
            tc.tile_pool(name="psR", bufs=3, space="PSUM") as psR,
            tc.tile_pool(name="psY", bufs=1, space="PSUM") as psY,
        ):
            # ---- load weights
            def wload(dram, shape, dtype):
                t = wts.tile(shape, dtype, tag=dram.name + "_s", name=dram.name + "_s")
                nc.sync.dma_start(t[:, :], dram[:, :])
                return t

            s_wxz = wload(wxz, [DM, 288], BF16)
            s_cd = wload(conv_diag, [DH, 18 * DH], BF16)
            s_cb = wload(conv_bias, [DH, 2], F32)
            s_xp = wload(xp_T, [DH, K * 2 * 80], BF16)
            s_dtw = wload(dtw_T, [R, K * DH], BF16)
            s_dtb = wload(dt_bias, [DH, K], F32)
            s_sc = wload(scales, [128, K * G], F32)
            s_wi8 = wload(wi8, [DH, G * 128], BF16)
            s_wbc = wload(wbc, [80, 128], BF16)
            s_wr = wload(wr12, [128, G * DH], BF16)
            s_ds = wload(ds_diag, [DH, K * DH], BF16)
            s_wout = wload(wout_T, [DH, DM], BF16)

            # matmul with optional LDWEIGHTS elision (stationary tensor is
            # already resident from the previous matmul)
            def mm(out, w, m, start=None, stop=None, ldw=True):
                inst = nc.tensor.matmul(out, w, m, start=start, stop=stop)
                if not ldw:
                    inst.ldweights = False
                return inst

            # front-end PSUM ring: rotate through the 5 psY banks
            _fe = [0]

            def fe_ps(rows):
                t = psY.tile([rows, 480], F32, tag=f"psY{_fe[0] % 5}",
                             name=f"fe{_fe[0]}")
                _fe[0] += 1
                return t

            # ---- x (host-transposed): [96, L] bf16
            xT = big.tile([DM, L], BF16, tag="xT")
            for h0, h1 in [(0, 1152), (1152, L)]:
                nc.sync.dma_start(xT[:, h0:h1], xt_in[:, h0:h1])

            # ---- pads for conv (one per half), zeroed borders (gpsimd: runs
            # once, before any DVE work it could contend with)
            pads = [big.tile([DH, 50 * 50], BF16, tag=f"pad{h}", name=f"pad{h}")
                    for h in range(2)]
            for p in pads:
                nc.gpsimd.memset(p[:, :], 0.0)

            # ---- in_proj: xc (both halves, into pad layout) + z half.
            # weight-outer so each of the 3 stationary tensors loads once.
            zs = big.tile([DH, L], BF16, tag="zs")
            for hh in range(2):
                for it, (t0, t1) in enumerate(TILES):
                    tw = t1 - t0
                    rows = tw // 48
                    ps = fe_ps(DH)
                    mm(ps[:, :tw], s_wxz[:, 96 * hh : 96 * hh + 96],
                       xT[:, t0:t1], ldw=(it == 0))
                    dst = pads[hh][:, :].rearrange("p (r c) -> p r c", r=50, c=50)[
                        :, 1 + 10 * it : 1 + 10 * it + rows, 1:49
                    ]
                    src = ps[:, :tw].rearrange("p (r c) -> p r c", r=rows, c=48)
                    nc.scalar.copy(dst, src)
            for it, (t0, t1) in enumerate(TILES):
                tw = t1 - t0
                ps = fe_ps(DH)
                mm(ps[:, :tw], s_wxz[:, 192:288], xT[:, t0:t1], ldw=(it == 0))
                nc.scalar.activation(zs[:, t0:t1], ps[:, :tw], AF.Silu)

            # ---- depthwise conv 3x3 + bias + silu -> u (per half), bf16.
            # row-block sets of 3 held in PSUM, tap-outer: 1 ldweights per
            # (half, set, tap) instead of per matmul.
            us = [big.tile([DH, L], BF16, tag=f"u{h}", name=f"u{h}") for h in range(2)]
            for st in range(2):
                for hh in range(2):
                    rbs = [3 * st, 3 * st + 1, 3 * st + 2]
                    pss = [fe_ps(DH) for _ in rbs]
                    j = 0
                    for dy in range(3):
                        for dx in range(3):
                            for i, rb in enumerate(rbs):
                                src = pads[hh][:, :].rearrange(
                                    "p (r c) -> p r c", r=50, c=50
                                )[:, 8 * rb + dy : 8 * rb + dy + 8, dx : dx + 48]
                                mm(pss[i][:, :384],
                                   s_cd[:, (hh * 9 + j) * DH : (hh * 9 + j + 1) * DH],
                                   src, start=(j == 0), stop=(j == 8),
                                   ldw=(i == 0))
                            j += 1
                    for i, rb in enumerate(rbs):
                        nc.scalar.activation(
                            us[hh][:, rb * 384 : rb * 384 + 384],
                            pss[i][:, :384],
                            AF.Silu,
                            bias=s_cb[:, hh : hh + 1],
                        )

            # ---- x_dbl per direction: [80, L] bf16, sections dt@0 B@32 C@64.
            # half-outer with 5 tiles live: 2 ldweights per direction.
            xdbls = [big.tile([80, L], BF16, tag=f"xdbl{k}", name=f"xdbl{k}")
                     for k in range(K)]
            for k in range(K):
                pss = [fe_ps(80) for _ in TILES]
                for hh in range(2):
                    for it, (t0, t1) in enumerate(TILES):
                        tw = t1 - t0
                        mm(pss[it][:, :tw],
                           s_xp[:, (2 * k + hh) * 80 : (2 * k + hh + 1) * 80],
                           us[hh][:, t0:t1], start=(hh == 0), stop=(hh == 1),
                           ldw=(it == 0))
                for it, (t0, t1) in enumerate(TILES):
                    nc.scalar.copy(xdbls[k][:, t0:t1], pss[it][:, : t1 - t0])

            # ---------------- per-direction prep (emitted in slices)
            def emit_prep_xk(k):
                if k == 0:
                    xk = xdbls[0]
                    u_k = us[0]
                elif k == 1:
                    xk = perd.tile([80, L], BF16, tag="xkp")
                    src = xdbls[1][:, :].rearrange("p (h w) -> p w h", h=H, w=W)
                    nc.scalar.copy(
                        xk[:, :].rearrange("p (a b) -> p a b", a=W, b=H), src)
                    u_k = perd.tile([DH, L], BF16, tag="ukp")
                    src = us[0][:, :].rearrange("p (h w) -> p w h", h=H, w=W)
                    nc.scalar.copy(
                        u_k[:, :].rearrange("p (a b) -> p a b", a=W, b=H), src)
                else:
                    # direction 2 runs the scan backwards over forward data
                    # (reversed APs); no permuted copies needed.
                    xk = xdbls[2]
                    u_k = us[0]
                return dict(xk=xk, u_k=u_k)

            def emit_prep_bc(P, sec):
                # B/C partition-broadcast (n-minor): [128, L] bf16 via PE
                xk = P["xk"]
                dstt = perd.tile([128, L], BF16,
                                 tag=("b_b" if sec == 32 else "c_b"))
                for it, (t0, t1) in enumerate(TILES):
                    tw = t1 - t0
                    psb = psR.tile([128, 480], F32, tag="psR")
                    mm(psb[:, :tw], s_wbc[sec : sec + 16, :],
                       xk[sec : sec + 16, t0:t1], ldw=(it == 0))
                    nc.scalar.copy(dstt[:, t0:t1], psb[:, :tw])
                return dstt

            def emit_prep_delta(k, P):
                # delta = ln(exp(v)+1), v = dtw @ dts + bias; bf16 [96, L]
                xk = P["xk"]
                delta = perd.tile([DH, L], BF16, tag="delta")
                for it, (t0, t1) in enumerate(TILES):
                    tw = t1 - t0
                    psv = psR.tile([DH, 480], F32, tag="psR")
                    mm(psv[:, :tw], s_dtw[:, k * DH : (k + 1) * DH],
                       xk[0:R, t0:t1], ldw=(it == 0))
                    ev = grp.tile([DH, 480], F32, tag="ev")
                    nc.scalar.activation(
                        ev[:, :tw], psv[:, :tw], AF.Exp,
                        bias=s_dtb[:, k : k + 1],
                    )
                    nc.scalar.activation(delta[:, t0:t1], ev[:, :tw], AF.Ln,
                                         bias=1.0)
                return delta

            def emit_prep_du(k, parts):
                P = parts[0]
                b_b, c_b, delta = parts[1], parts[2], parts[3]
                du = perd.tile([DH, L], BF16, tag="du")
                nc.vector.tensor_mul(du[:, :], delta[:, :], P["u_k"][:, :])
                return dict(xk=P["xk"], u_k=P["u_k"], b_b=b_b, c_b=c_b,
                            delta=delta, du=du)

            def emit_prep(k):
                P = emit_prep_xk(k)
                b_b = emit_prep_bc(P, 32)
                c_b = emit_prep_bc(P, 64)
                delta = emit_prep_delta(k, P)
                return emit_prep_du(k, [P, b_b, c_b, delta])

            # ---------------- directions
            ysb = [big.tile([DH, L], BF16, tag=f"ysb{k}", name=f"ysb{k}")
                   for k in range(K)]
            prep = emit_prep(0)
            for k in range(K):
                P = prep
                u_k, b_b, c_b, delta, du = (P["u_k"], P["b_b"], P["c_b"],
                                            P["delta"], P["du"])
                psy = [psY.tile([DH, 480], F32, tag=f"psY{c}", name=f"psy{c}_{k}")
                       for c in range(len(TILES))]

                pend = None   # (g, ch) awaiting y-reduction
                for g in range(G):
                    # PE replication [96 -> 128=(d8,n16)] via 0/1 matmuls;
                    # wi8_g stays resident for all 10 chunk-matmuls.
                    a_t = grp.tile([128, L], BF16, tag="a")
                    dub = grp.tile([128, L], BF16, tag="dub")
                    first = True
                    for t0, t1 in TILES:
                        tw = t1 - t0
                        psr = psR.tile([128, 480], F32, tag="psR")
                        mm(psr[:, :tw], s_wi8[:, g * 128 : (g + 1) * 128],
                           delta[:, t0:t1], ldw=first)
                        first = False
                        nc.scalar.activation(
                            a_t[:, t0:t1], psr[:, :tw], AF.Exp,
                            scale=s_sc[:, k * G + g : k * G + g + 1],
                        )
                        psd = psR.tile([128, 480], F32, tag="psR")
                        mm(psd[:, :tw], s_wi8[:, g * 128 : (g + 1) * 128],
                           du[:, t0:t1], ldw=False)
                        nc.scalar.copy(dub[:, t0:t1], psd[:, :tw])
                    # w = (delta*u) * B
                    w_t = grp.tile([128, L], BF16, tag="w")
                    nc.vector.tensor_mul(w_t[:, :], dub[:, :], b_b[:, :])
                    # scan along t (2 chained halves; dir 2 scans backwards
                    # through reversed views so h lands in forward order)
                    h_t = grp.tile([128, L], BF16, tag="h")
                    if k < 2:
                        nc.vector.tensor_tensor_scan(
                            h_t[:, 0:1152], a_t[:, 0:1152], w_t[:, 0:1152],
                            0.0, MUL, ADD,
                        )
                        nc.vector.tensor_tensor_scan(
                            h_t[:, 1152:L], a_t[:, 1152:L], w_t[:, 1152:L],
                            h_t[:, 1151:1152], MUL, ADD,
                        )
                    else:
                        nc.vector.tensor_tensor_scan(
                            h_t[:, 1152:L][:, ::-1], a_t[:, 1152:L][:, ::-1],
                            w_t[:, 1152:L][:, ::-1], 0.0, MUL, ADD,
                        )
                        nc.vector.tensor_tensor_scan(
                            h_t[:, 0:1152][:, ::-1], a_t[:, 0:1152][:, ::-1],
                            w_t[:, 0:1152][:, ::-1], h_t[:, 1152:1153],
                            MUL, ADD,
                        )
                    # ch = h * C
                    ch = grp.tile([128, L], BF16, tag="ch")
                    nc.vector.tensor_mul(ch[:, :], h_t[:, :], c_b[:, :])
                    # y-reduce one group behind so PE never waits on ch
                    if pend is not None:
                        pg, pch = pend
                        for c, (t0, t1) in enumerate(TILES):
                            mm(psy[c][:, : t1 - t0],
                               s_wr[:, pg * DH : (pg + 1) * DH],
                               pch[:, t0:t1], start=(pg == 0), stop=False,
                               ldw=(c == 0))
                    pend = (g, ch)
                    # prefetch next direction's prep in slices
                    if k + 1 < K:
                        if g == 1:
                            prep_parts = [emit_prep_xk(k + 1)]
                        elif g == 3:
                            prep_parts.append(emit_prep_bc(prep_parts[0], 32))
                        elif g == 5:
                            prep_parts.append(emit_prep_bc(prep_parts[0], 64))
                        elif g == 7:
                            prep_parts.append(emit_prep_delta(k + 1,
                                                             prep_parts[0]))
                        elif g == 9:
                            prep = emit_prep_du(k + 1, prep_parts)
                # flush the deferred y-reduction
                pg, pch = pend
                for c, (t0, t1) in enumerate(TILES):
                    mm(psy[c][:, : t1 - t0],
                       s_wr[:, pg * DH : (pg + 1) * DH],
                       pch[:, t0:t1], start=(pg == 0), stop=False,
                       ldw=(c == 0))
                # y += Ds * u  (diagonal matmul closes the accumulation)
                for c, (t0, t1) in enumerate(TILES):
                    mm(psy[c][:, : t1 - t0],
                       s_ds[:, k * DH : (k + 1) * DH],
                       u_k[:, t0:t1], start=False, stop=True, ldw=(c == 0))
                for c, (t0, t1) in enumerate(TILES):
                    nc.scalar.copy(ysb[k][:, t0:t1], psy[c][:, : t1 - t0])

            # ---- merge directions (undo orderings, in-place in ysb0), gate
            p1 = ysb[1][:, :].rearrange("p (w h) -> p h w", w=W, h=H)
            nc.vector.tensor_add(
                ysb[0][:, :].rearrange("p (a b) -> p a b", a=H, b=W),
                ysb[0][:, :].rearrange("p (a b) -> p a b", a=H, b=W),
                p1,
            )
            nc.vector.tensor_add(ysb[0][:, :], ysb[0][:, :], ysb[2][:, :])
            nc.vector.tensor_mul(ysb[0][:, :], ysb[0][:, :], zs[:, :])

            out_sb = big.tile([DM, L], F32, tag="pad0", name="out_sb")
            for it, (t0, t1) in enumerate(TILES):
                tw = t1 - t0
                ps = psR.tile([DM, 480], F32, tag="psR")
                mm(ps[:, :tw], s_wout[:, :], ysb[0][:, t0:t1], ldw=(it == 0))
                nc.scalar.copy(out_sb[:, t0:t1], ps[:, :tw])
                nc.sync.dma_start(out_part[:, t0:t1], out_sb[:, t0:t1])

    return nc


def _prep_in_maps(inputs):
    f32 = lambda a: np.ascontiguousarray(np.asarray(a, np.float32))
    x = f32(inputs["x"])
    in_proj_w = f32(inputs["in_proj_w"])        # (384, 96)
    conv_w = f32(inputs["conv_w"]).reshape(DI, 9)
    conv_b = f32(inputs["conv_b"])
    x_proj_w = f32(inputs["x_proj_w"])          # (K, 38, 192)
    dt_w = f32(inputs["dt_projs_w"])            # (K, 192, 6)
    dt_b = f32(inputs["dt_projs_b"])            # (K, 192)
    A = -np.exp(f32(inputs["A_logs"])).reshape(K, DI, N)
    Ds = f32(inputs["Ds"]).reshape(K, DI)
    out_w = f32(inputs["out_proj_w"])           # (96, 192)

    # y-reduction: psy[d'] = sum_p wr[p, d'] * ch[p];  p = d8*16 + n
    wr_np = np.zeros((128, G * DH), np.float32)
    for g in range(G):
        for d8 in range(8):
            wr_np[d8 * 16 : d8 * 16 + 16, g * DH + g * 8 + d8] = 1.0

    # group-select replication: wi8[g*8+d8, g*128 + d8*16 + n] = 1
    wi8_np = np.zeros((DH, G * 128), np.float32)
    for g in range(G):
        for d8 in range(8):
            wi8_np[g * 8 + d8, g * 128 + d8 * 16 : g * 128 + d8 * 16 + 16] = 1.0

    # B/C broadcast: row 32+n (or 64+n) -> partitions d8*16+n
    wbc_np = np.zeros((80, 128), np.float32)
    for n in range(16):
        wbc_np[32 + n, n::16] = 1.0
        wbc_np[64 + n, n::16] = 1.0

    in_maps = []
    for c in range(8):
        b, half = c // 2, c % 2
        pd = np.concatenate([np.arange(DI)[96 * half : 96 * half + 96],
                             np.arange(DI)[96 * (1 - half) : 96 * (1 - half) + 96]])
        dh = pd[:DH]

        wxz_np = np.zeros((DM, 288), np.float32)
        wxz_np[:, 0:96] = in_proj_w[pd[:96]].T
        wxz_np[:, 96:192] = in_proj_w[pd[96:]].T
        wxz_np[:, 192:288] = in_proj_w[DI + dh].T

        cd = np.zeros((DH, 18 * DH), np.float32)
        for hh in range(2):
            ch_idx = pd[hh * 96 : hh * 96 + 96]
            for j in range(9):
                blk = np.zeros((DH, DH), np.float32)
                np.fill_diagonal(blk, conv_w[ch_idx, j])
                cd[:, (hh * 9 + j) * DH : (hh * 9 + j + 1) * DH] = blk
        cb = np.stack([conv_b[pd[:96]], conv_b[pd[96:]]], axis=1)

        xp = np.zeros((DH, K * 2 * 80), np.float32)
        for k in range(K):
            for hh in range(2):
                blk = np.zeros((DH, 80), np.float32)
                ch_idx = pd[hh * 96 : hh * 96 + 96]
                blk[:, 0:6] = x_proj_w[k][0:6, ch_idx].T
                blk[:, 32:48] = x_proj_w[k][6:22, ch_idx].T
                blk[:, 64:80] = x_proj_w[k][22:38, ch_idx].T
                xp[:, (2 * k + hh) * 80 : (2 * k + hh + 1) * 80] = blk

        dtw = np.zeros((R, K * DH), np.float32)
        for k in range(K):
            dtw[:, k * DH : (k + 1) * DH] = dt_w[k][dh].T
        dtb = np.stack([dt_b[k][dh] for k in range(K)], axis=1)

        # per-partition decay scales: scales[p, k*G+g] = A[k, dh[g*8+d8], n]
        # with p = d8*16 + n
        sc = np.zeros((128, K * G), np.float32)
        for k in range(K):
            for g in range(G):
                for d8 in range(8):
                    sc[d8 * 16 : d8 * 16 + 16, k * G + g] = A[k, dh[g * 8 + d8]]

        dsd = np.zeros((DH, K * DH), np.float32)
        for k in range(K):
            np.fill_diagonal(dsd[:, k * DH : (k + 1) * DH], Ds[k][dh])

        bf = ml_dtypes.bfloat16
        in_maps.append(
            dict(
                xt_in=np.ascontiguousarray(x[b].reshape(L, DM).T).astype(bf),
                wxz=wxz_np.astype(bf),
                conv_diag=cd.astype(bf),
                conv_bias=np.ascontiguousarray(cb),
                xp_T=xp.astype(bf),
                dtw_T=dtw.astype(bf),
                dt_bias=np.ascontiguousarray(dtb),
                scales=sc,
                wi8=wi8_np.astype(bf),
                wbc=wbc_np.astype(bf),
                wr12=wr_np.astype(bf),
                ds_diag=dsd.astype(bf),
                wout_T=np.ascontiguousarray(out_w[:, dh].T).astype(bf),
            )
        )
    return in_maps


def kernel(**inputs):
    if "nc" not in _COMPILED:
        _COMPILED["nc"] = _build_nc()
    nc = _COMPILED["nc"]
    in_maps = _prep_in_maps(inputs)
    res = run_bass_kernel_spmd(nc, in_maps, core_ids=list(range(8)))
    out = np.zeros((B, H, W, DM), np.float32)
    for b in range(B):
        p = res.results[2 * b]["out_part"] + res.results[2 * b + 1]["out_part"]
        out[b] = p.T.reshape(H, W, DM)
    return out


# revision 8
# speedup vs baseline: 1.0046x; 1.0046x over previous
"""DSSM (vision Mamba block) Trainium2 kernel — v4.

Problem: B=4, H=W=48, L=2304, D_MODEL=96, D_INNER=192, N=16, R=6, K=3.

Sharding (8 cores, no device-to-device comms):
  core c -> batch b=c//2, d_inner half=c%2. Each core runs the full-d
  front-end (in_proj, depthwise conv, x_dbl) for its batch, the K=3
  selective scans for its 96 d_inner channels, and a partial out_proj
  (contraction over its d-half). Host sums the two partials per batch.

Design notes (measured on HW):
  - DVE scan = ~2.0 ns/col and is the irreducible bottleneck
    (36 groups x [128,2304]); everything else hides under it.
  - x is transposed to [96, L] bf16 on the HOST (the row-major on-device
    load cost ~115us of per-row DMA descriptors).
  - Delta / Delta*u are partition-replicated [96 -> 128=(d8,n16)] by PE
    0/1 matmuls; ACT consumes PSUM directly: a = exp(A*Delta) with a
    per-partition scale vector (exact A), dub copied to SBUF bf16 so the
    w-mul keeps the 2x bf16 DVE rate.
  - all matmuls bf16 (fp32 emits 2 half-speed instructions).
  - LDWEIGHTS elided (inst.ldweights=False) wherever consecutive matmuls
    share a stationary tensor; loops are ordered weight-outer for this.
  - front-end shares the 5 psY PSUM banks as its pipeline ring; psR
    (bufs=3) serves the per-direction streams. 5 + 3 = 8 banks.
  - GpSimd is avoided for bulk work: it contends with DVE SBUF ports
    (+75% measured on concurrent DVE ops).
"""

import numpy as np
import ml_dtypes

import concourse.bass as bass
import concourse.mybir as mybir
import concourse.tile as tile
from concourse.bass_utils import run_bass_kernel_spmd

# ---------------------------------------------------------------- tile fix
# The walrus here accepts only ONE inline sem-wait per instruction; Tile can
# attach several. Hoist extras onto same-engine NOPs placed just before.
_wsplit_counter = [0]


def _split_multi_waits(nc):
    for fn in nc.m.functions:
        for blk in fn.blocks:
            out = []
            changed = False
            for inst in blk.instructions:
                si = inst.sync_info
                waits = list(si.on_wait) if si is not None and si.on_wait else []
                if len(waits) > 1:
                    changed = True
                    for w in waits[:-1]:
                        _wsplit_counter[0] += 1
                        nop = mybir.InstNoOp(name=f"wsplit-{_wsplit_counter[0]}")
                        nop.engine = inst.engine
                        nop.sync_info = mybir.SyncInfo(on_wait=[w], on_update=[])
                        out.append(nop)
                    inst.sync_info = mybir.SyncInfo(
                        on_wait=[waits[-1]],
                        on_update=list(si.on_update) if si.on_update else [],
                    )
                out.append(inst)
            if changed:
                blk.instructions = out


class TileContextFixed(tile.TileContext):
    def __exit__(self, exc_type, exc_val, exc_tb):
        r = super().__exit__(exc_type, exc_val, exc_tb)
        if exc_type is None:
            _split_multi_waits(self.nc)
        return r


# ---------------------------------------------------------------- constants
B, H, W = 4, 48, 48
DM, DI, N, R, K = 96, 192, 16, 6, 3
L = H * W
DH = 96          # d-half per core
G = DH // 8      # 12 groups of 8 channels
TILES = [(0, 480), (480, 960), (960, 1440), (1440, 1920), (1920, 2304)]

F32 = mybir.dt.float32
BF16 = mybir.dt.bfloat16
MUL = mybir.AluOpType.mult
ADD = mybir.AluOpType.add
AF = mybir.ActivationFunctionType

_COMPILED = {}


def _build_nc():
    nc = bass.Bass()

    # ---- dram I/O (per-core values supplied via in_maps)
    xt_in = nc.dram_tensor("xt_in", [DM, L], BF16, kind="ExternalInput")
    wxz = nc.dram_tensor("wxz", [DM, 288], BF16, kind="ExternalInput")
    conv_diag = nc.dram_tensor("conv_diag", [DH, 18 * DH], BF16, kind="ExternalInput")
    conv_bias = nc.dram_tensor("conv_bias", [DH, 2], F32, kind="ExternalInput")
    xp_T = nc.dram_tensor("xp_T", [DH, K * 2 * 80], BF16, kind="ExternalInput")
    dtw_T = nc.dram_tensor("dtw_T", [R, K * DH], BF16, kind="ExternalInput")
    dt_bias = nc.dram_tensor("dt_bias", [DH, K], F32, kind="ExternalInput")
    scales = nc.dram_tensor("scales", [128, K * G], F32, kind="ExternalInput")
    wi8 = nc.dram_tensor("wi8", [DH, G * 128], BF16, kind="ExternalInput")
    wbc = nc.dram_tensor("wbc", [80, 128], BF16, kind="ExternalInput")
    wr12 = nc.dram_tensor("wr12", [128, G * DH], BF16, kind="ExternalInput")
    ds_diag = nc.dram_tensor("ds_diag", [DH, K * DH], BF16, kind="ExternalInput")
    wout_T = nc.dram_tensor("wout_T", [DH, DM], BF16, kind="ExternalInput")
    out_part = nc.dram_tensor("out_part", [DM, L], F32, kind="ExternalOutput")

    with TileContextFixed(nc) as tc:
        with (
            tc.tile_pool(name="wts", bufs=1) as wts,
            tc.tile_pool(name="big", bufs=1) as big,
            tc.tile_pool(name="perd", bufs=2) as perd,
            tc.tile_pool(name="grp", bufs=2) as grp,
            tc.tile_pool(name="psR", bufs=3, space="PSUM") as psR,
            tc.tile_pool(name="psY", bufs=1, space="PSUM") as psY,
        ):
            # ---- load weights
            def wload(dram, shape, dtype):
                t = wts.tile(shape, dtype, tag=dram.name + "_s", name=dram.name + "_s")
                nc.sync.dma_start(t[:, :], dram[:, :])
                return t

            s_wxz = wload(wxz, [DM, 288], BF16)
            s_cd = wload(conv_diag, [DH, 18 * DH], BF16)
            s_cb = wload(conv_bias, [DH, 2], F32)
            s_xp = wload(xp_T, [DH, K * 2 * 80], BF16)
            s_dtw = wload(dtw_T, [R, K * DH], BF16)
            s_dtb = wload(dt_bias, [DH, K], F32)
            s_sc = wload(scales, [128, K * G], F32)
            s_wi8 = wload(wi8, [DH, G * 128], BF16)
            s_wbc = wload(wbc, [80, 128], BF16)
            s_wr = wload(wr12, [128, G * DH], BF16)
            s_ds = wload(ds_diag, [DH, K * DH], BF16)
            s_wout = wload(wout_T, [DH, DM], BF16)

            # matmul with optional LDWEIGHTS elision (stationary tensor is
            # already resident from the previous matmul)
            def mm(out, w, m, start=None, stop=None, ldw=True):
                inst = nc.tensor.matmul(out, w, m, start=start, stop=stop)
                if not ldw:
                    inst.ldweights = False
                return inst

            # front-end PSUM ring: rotate through the 5 psY banks
            _fe = [0]

            def fe_ps(rows):
                t = psY.tile([rows, 480], F32, tag=f"psY{_fe[0] % 5}",
                             name=f"fe{_fe[0]}")
                _fe[0] += 1
                return t

            # ---- x (host-transposed): [96, L] bf16
            xT = big.tile([DM, L], BF16, tag="xT")
            for h0, h1 in [(0, 1152), (1152, L)]:
                nc.sync.dma_start(xT[:, h0:h1], xt_in[:, h0:h1])

            # ---- pads for conv (one per half), zeroed borders (gpsimd: runs
            # once, before any DVE work it could contend with)
            pads = [big.tile([DH, 50 * 50], BF16, tag=f"pad{h}", name=f"pad{h}")
                    for h in range(2)]
            for p in pads:
                nc.gpsimd.memset(p[:, :], 0.0)

            # ---- in_proj: xc (both halves, into pad layout) + z half.
            # weight-outer so each of the 3 stationary tensors loads once.
            zs = big.tile([DH, L], BF16, tag="zs")
            for hh in range(2):
                for it, (t0, t1) in enumerate(TILES):
                    tw = t1 - t0
                    rows = tw // 48
                    ps = fe_ps(DH)
                    mm(ps[:, :tw], s_wxz[:, 96 * hh : 96 * hh + 96],
                       xT[:, t0:t1], ldw=(it == 0))
                    dst = pads[hh][:, :].rearrange("p (r c) -> p r c", r=50, c=50)[
                        :, 1 + 10 * it : 1 + 10 * it + rows, 1:49
                    ]
                    src = ps[:, :tw].rearrange("p (r c) -> p r c", r=rows, c=48)
                    nc.scalar.copy(dst, src)

            # ---- depthwise conv 3x3 + bias + silu -> u (per half), bf16.
            # row-block sets of 3 held in PSUM, tap-outer: 1 ldweights per
            # (half, set, tap) instead of per matmul.
            us = [big.tile([DH, L], BF16, tag=f"u{h}", name=f"u{h}") for h in range(2)]
            for st in range(2):
                for hh in range(2):
                    rbs = [3 * st, 3 * st + 1, 3 * st + 2]
                    pss = [fe_ps(DH) for _ in rbs]
                    j = 0
                    for dy in range(3):
                        for dx in range(3):
                            for i, rb in enumerate(rbs):
                                src = pads[hh][:, :].rearrange(
                                    "p (r c) -> p r c", r=50, c=50
                                )[:, 8 * rb + dy : 8 * rb + dy + 8, dx : dx + 48]
                                mm(pss[i][:, :384],
                                   s_cd[:, (hh * 9 + j) * DH : (hh * 9 + j + 1) * DH],
                                   src, start=(j == 0), stop=(j == 8),
                                   ldw=(i == 0))
                            j += 1
                    for i, rb in enumerate(rbs):
                        nc.scalar.activation(
                            us[hh][:, rb * 384 : rb * 384 + 384],
                            pss[i][:, :384],
                            AF.Silu,
                            bias=s_cb[:, hh : hh + 1],
                        )

            # ---- x_dbl per direction: [80, L] bf16, sections dt@0 B@32 C@64.
            # half-outer with 5 tiles live: 2 ldweights per direction.
            xdbls = [big.tile([80, L], BF16, tag=f"xdbl{k}", name=f"xdbl{k}")
                     for k in range(K)]

            def emit_xdbl(k):
                pss = [fe_ps(80) for _ in TILES]
                for hh in range(2):
                    for it, (t0, t1) in enumerate(TILES):
                        tw = t1 - t0
                        mm(pss[it][:, :tw],
                           s_xp[:, (2 * k + hh) * 80 : (2 * k + hh + 1) * 80],
                           us[hh][:, t0:t1], start=(hh == 0), stop=(hh == 1),
                           ldw=(it == 0))
                for it, (t0, t1) in enumerate(TILES):
                    nc.scalar.copy(xdbls[k][:, t0:t1], pss[it][:, : t1 - t0])

            def emit_z():
                for it, (t0, t1) in enumerate(TILES):
                    tw = t1 - t0
                    ps = fe_ps(DH)
                    mm(ps[:, :tw], s_wxz[:, 192:288], xT[:, t0:t1],
                       ldw=(it == 0))
                    nc.scalar.activation(zs[:, t0:t1], ps[:, :tw], AF.Silu)

            emit_xdbl(0)

            # ---------------- per-direction prep (emitted in slices)
            def emit_prep_xk(k):
                if k == 0:
                    xk = xdbls[0]
                    u_k = us[0]
                elif k == 1:
                    xk = perd.tile([80, L], BF16, tag="xkp")
                    src = xdbls[1][:, :].rearrange("p (h w) -> p w h", h=H, w=W)
                    nc.scalar.copy(
                        xk[:, :].rearrange("p (a b) -> p a b", a=W, b=H), src)
                    u_k = perd.tile([DH, L], BF16, tag="ukp")
                    src = us[0][:, :].rearrange("p (h w) -> p w h", h=H, w=W)
                    nc.scalar.copy(
                        u_k[:, :].rearrange("p (a b) -> p a b", a=W, b=H), src)
                else:
                    # direction 2 runs the scan backwards over forward data
                    # (reversed APs); no permuted copies needed.
                    xk = xdbls[2]
                    u_k = us[0]
                return dict(xk=xk, u_k=u_k)

            def emit_prep_bc(P, sec):
                # B/C partition-broadcast (n-minor): [128, L] bf16 via PE
                xk = P["xk"]
                dstt = perd.tile([128, L], BF16,
                                 tag=("b_b" if sec == 32 else "c_b"))
                for it, (t0, t1) in enumerate(TILES):
                    tw = t1 - t0
                    psb = psR.tile([128, 480], F32, tag="psR")
                    mm(psb[:, :tw], s_wbc[sec : sec + 16, :],
                       xk[sec : sec + 16, t0:t1], ldw=(it == 0))
                    nc.scalar.copy(dstt[:, t0:t1], psb[:, :tw])
                return dstt

            def emit_prep_delta(k, P):
                # delta = ln(exp(v)+1), v = dtw @ dts + bias; bf16 [96, L]
                xk = P["xk"]
                delta = perd.tile([DH, L], BF16, tag="delta")
                for it, (t0, t1) in enumerate(TILES):
                    tw = t1 - t0
                    psv = psR.tile([DH, 480], F32, tag="psR")
                    mm(psv[:, :tw], s_dtw[:, k * DH : (k + 1) * DH],
                       xk[0:R, t0:t1], ldw=(it == 0))
                    ev = grp.tile([DH, 480], F32, tag="ev")
                    nc.scalar.activation(
                        ev[:, :tw], psv[:, :tw], AF.Exp,
                        bias=s_dtb[:, k : k + 1],
                    )
                    nc.scalar.activation(delta[:, t0:t1], ev[:, :tw], AF.Ln,
                                         bias=1.0)
                return delta

            def emit_prep_du(k, parts):
                P = parts[0]
                b_b, c_b, delta = parts[1], parts[2], parts[3]
                du = perd.tile([DH, L], BF16, tag="du")
                nc.vector.tensor_mul(du[:, :], delta[:, :], P["u_k"][:, :])
                return dict(xk=P["xk"], u_k=P["u_k"], b_b=b_b, c_b=c_b,
                            delta=delta, du=du)

            def emit_prep(k):
                P = emit_prep_xk(k)
                b_b = emit_prep_bc(P, 32)
                c_b = emit_prep_bc(P, 64)
                delta = emit_prep_delta(k, P)
                return emit_prep_du(k, [P, b_b, c_b, delta])

            # ---------------- directions
            ysb = [big.tile([DH, L], BF16, tag=f"ysb{k}", name=f"ysb{k}")
                   for k in range(K)]
            prep = emit_prep(0)
            for k in range(K):
                P = prep
                u_k, b_b, c_b, delta, du = (P["u_k"], P["b_b"], P["c_b"],
                                            P["delta"], P["du"])
                psy = [psY.tile([DH, 480], F32, tag=f"psY{c}", name=f"psy{c}_{k}")
                       for c in range(len(TILES))]

                pend = None   # (g, ch) awaiting y-reduction
                for g in range(G):
                    # PE replication [96 -> 128=(d8,n16)] via 0/1 matmuls;
                    # wi8_g stays resident for all 10 chunk-matmuls.
                    a_t = grp.tile([128, L], BF16, tag="a")
                    dub = grp.tile([128, L], BF16, tag="dub")
                    first = True
                    for t0, t1 in TILES:
                        tw = t1 - t0
                        psr = psR.tile([128, 480], F32, tag="psR")
                        mm(psr[:, :tw], s_wi8[:, g * 128 : (g + 1) * 128],
                           delta[:, t0:t1], ldw=first)
                        first = False
                        nc.scalar.activation(
                            a_t[:, t0:t1], psr[:, :tw], AF.Exp,
                            scale=s_sc[:, k * G + g : k * G + g + 1],
                        )
                        psd = psR.tile([128, 480], F32, tag="psR")
                        mm(psd[:, :tw], s_wi8[:, g * 128 : (g + 1) * 128],
                           du[:, t0:t1], ldw=False)
                        nc.scalar.copy(dub[:, t0:t1], psd[:, :tw])
                    # w = (delta*u) * B
                    w_t = grp.tile([128, L], BF16, tag="w")
                    nc.vector.tensor_mul(w_t[:, :], dub[:, :], b_b[:, :])
                    # scan along t (2 chained halves; dir 2 scans backwards
                    # through reversed views so h lands in forward order)
                    h_t = grp.tile([128, L], BF16, tag="h")
                    if k < 2:
                        nc.vector.tensor_tensor_scan(
                            h_t[:, 0:1152], a_t[:, 0:1152], w_t[:, 0:1152],
                            0.0, MUL, ADD,
                        )
                        nc.vector.tensor_tensor_scan(
                            h_t[:, 1152:L], a_t[:, 1152:L], w_t[:, 1152:L],
                            h_t[:, 1151:1152], MUL, ADD,
                        )
                    else:
                        nc.vector.tensor_tensor_scan(
                            h_t[:, 1152:L][:, ::-1], a_t[:, 1152:L][:, ::-1],
                            w_t[:, 1152:L][:, ::-1], 0.0, MUL, ADD,
                        )
                        nc.vector.tensor_tensor_scan(
                            h_t[:, 0:1152][:, ::-1], a_t[:, 0:1152][:, ::-1],
                            w_t[:, 0:1152][:, ::-1], h_t[:, 1152:1153],
                            MUL, ADD,
                        )
                    # ch = h * C
                    ch = grp.tile([128, L], BF16, tag="ch")
                    nc.vector.tensor_mul(ch[:, :], h_t[:, :], c_b[:, :])
                    # y-reduce one group behind so PE never waits on ch
                    if pend is not None:
                        pg, pch = pend
                        for c, (t0, t1) in enumerate(TILES):
                            mm(psy[c][:, : t1 - t0],
                               s_wr[:, pg * DH : (pg + 1) * DH],
                               pch[:, t0:t1], start=(pg == 0), stop=False,
                               ldw=(c == 0))
                    pend = (g, ch)
                    # deferred front-end pieces hide inside dir-0's stream
                    if k == 0:
                        if g == 0:
                            emit_xdbl(1)
                        elif g == 2:
                            emit_xdbl(2)
                        elif g == 10:
                            emit_z()
                    # prefetch next direction's prep in slices
                    if k + 1 < K:
                        if g == 3:
                            prep_parts = [emit_prep_xk(k + 1)]
                        elif g == 5:
                            prep_parts.append(emit_prep_bc(prep_parts[0], 32))
                        elif g == 6:
                            prep_parts.append(emit_prep_bc(prep_parts[0], 64))
                        elif g == 8:
                            prep_parts.append(emit_prep_delta(k + 1,
                                                             prep_parts[0]))
                        elif g == 10:
                            prep = emit_prep_du(k + 1, prep_parts)
                # flush the deferred y-reduction
                pg, pch = pend
                for c, (t0, t1) in enumerate(TILES):
                    mm(psy[c][:, : t1 - t0],
                       s_wr[:, pg * DH : (pg + 1) * DH],
                       pch[:, t0:t1], start=(pg == 0), stop=False,
                       ldw=(c == 0))
                # y += Ds * u  (diagonal matmul closes the accumulation)
                for c, (t0, t1) in enumerate(TILES):
                    mm(psy[c][:, : t1 - t0],
                       s_ds[:, k * DH : (k + 1) * DH],
                       u_k[:, t0:t1], start=False, stop=True, ldw=(c == 0))
                for c, (t0, t1) in enumerate(TILES):
                    nc.scalar.copy(ysb[k][:, t0:t1], psy[c][:, : t1 - t0])

            # ---- merge directions (undo orderings), gate, out — chunked
            # so the tail streams instead of serializing full-L passes
            out_sb = big.tile([DM, L], F32, tag="pad0", name="out_sb")
            p1full = ysb[1][:, :].rearrange("p (w h) -> p h w", w=W, h=H)
            y0v = ysb[0][:, :].rearrange("p (a b) -> p a b", a=H, b=W)
            for it, (t0, t1) in enumerate(TILES):
                tw = t1 - t0
                r0, r1 = t0 // W, t1 // W
                nc.vector.tensor_add(y0v[:, r0:r1, :], y0v[:, r0:r1, :],
                                     p1full[:, r0:r1, :])
                nc.vector.tensor_add(ysb[0][:, t0:t1], ysb[0][:, t0:t1],
                                     ysb[2][:, t0:t1])
                nc.vector.tensor_mul(ysb[0][:, t0:t1], ysb[0][:, t0:t1],
                                     zs[:, t0:t1])
                ps = psR.tile([DM, 480], F32, tag="psR")
                mm(ps[:, :tw], s_wout[:, :], ysb[0][:, t0:t1], ldw=(it == 0))
                nc.scalar.copy(out_sb[:, t0:t1], ps[:, :tw])
                nc.sync.dma_start(out_part[:, t0:t1], out_sb[:, t0:t1])

    return nc


def _prep_in_maps(inputs):
    f32 = lambda a: np.ascontiguousarray(np.asarray(a, np.float32))
    x = f32(inputs["x"])
    in_proj_w = f32(inputs["in_proj_w"])        # (384, 96)
    conv_w = f32(inputs["conv_w"]).reshape(DI, 9)
    conv_b = f32(inputs["conv_b"])
    x_proj_w = f32(inputs["x_proj_w"])          # (K, 38, 192)
    dt_w = f32(inputs["dt_projs_w"])            # (K, 192, 6)
    dt_b = f32(inputs["dt_projs_b"])            # (K, 192)
    A = -np.exp(f32(inputs["A_logs"])).reshape(K, DI, N)
    Ds = f32(inputs["Ds"]).reshape(K, DI)
    out_w = f32(inputs["out_proj_w"])           # (96, 192)

    # y-reduction: psy[d'] = sum_p wr[p, d'] * ch[p];  p = d8*16 + n
    wr_np = np.zeros((128, G * DH), np.float32)
    for g in range(G):
        for d8 in range(8):
            wr_np[d8 * 16 : d8 * 16 + 16, g * DH + g * 8 + d8] = 1.0

    # group-select replication: wi8[g*8+d8, g*128 + d8*16 + n] = 1
    wi8_np = np.zeros((DH, G * 128), np.float32)
    for g in range(G):
        for d8 in range(8):
            wi8_np[g * 8 + d8, g * 128 + d8 * 16 : g * 128 + d8 * 16 + 16] = 1.0

    # B/C broadcast: row 32+n (or 64+n) -> partitions d8*16+n
    wbc_np = np.zeros((80, 128), np.float32)
    for n in range(16):
        wbc_np[32 + n, n::16] = 1.0
        wbc_np[64 + n, n::16] = 1.0

    in_maps = []
    for c in range(8):
        b, half = c // 2, c % 2
        pd = np.concatenate([np.arange(DI)[96 * half : 96 * half + 96],
                             np.arange(DI)[96 * (1 - half) : 96 * (1 - half) + 96]])
        dh = pd[:DH]

        wxz_np = np.zeros((DM, 288), np.float32)
        wxz_np[:, 0:96] = in_proj_w[pd[:96]].T
        wxz_np[:, 96:192] = in_proj_w[pd[96:]].T
        wxz_np[:, 192:288] = in_proj_w[DI + dh].T

        cd = np.zeros((DH, 18 * DH), np.float32)
        for hh in range(2):
            ch_idx = pd[hh * 96 : hh * 96 + 96]
            for j in range(9):
                blk = np.zeros((DH, DH), np.float32)
                np.fill_diagonal(blk, conv_w[ch_idx, j])
                cd[:, (hh * 9 + j) * DH : (hh * 9 + j + 1) * DH] = blk
        cb = np.stack([conv_b[pd[:96]], conv_b[pd[96:]]], axis=1)

        xp = np.zeros((DH, K * 2 * 80), np.float32)
        for k in range(K):
            for hh in range(2):
                blk = np.zeros((DH, 80), np.float32)
                ch_idx = pd[hh * 96 : hh * 96 + 96]
                blk[:, 0:6] = x_proj_w[k][0:6, ch_idx].T
                blk[:, 32:48] = x_proj_w[k][6:22, ch_idx].T
                blk[:, 64:80] = x_proj_w[k][22:38, ch_idx].T
                xp[:, (2 * k + hh) * 80 : (2 * k + hh + 1) * 80] = blk

        dtw = np.zeros((R, K * DH), np.float32)
        for k in range(K):
            dtw[:, k * DH : (k + 1) * DH] = dt_w[k][dh].T
        dtb = np.stack([dt_b[k][dh] for k in range(K)], axis=1)

        # per-partition decay scales: scales[p, k*G+g] = A[k, dh[g*8+d8], n]
        # with p = d8*16 + n
        sc = np.zeros((128, K * G), np.float32)
        for k in range(K):
            for g in range(G):
                for d8 in range(8):
                    sc[d8 * 16 : d8 * 16 + 16, k * G + g] = A[k, dh[g * 8 + d8]]

        dsd = np.zeros((DH, K * DH), np.float32)
        for k in range(K):
            np.fill_diagonal(dsd[:, k * DH : (k + 1) * DH], Ds[k][dh])

        bf = ml_dtypes.bfloat16
        in_maps.append(
            dict(
                xt_in=np.ascontiguousarray(x[b].reshape(L, DM).T).astype(bf),
                wxz=wxz_np.astype(bf),
                conv_diag=cd.astype(bf),
                conv_bias=np.ascontiguousarray(cb),
                xp_T=xp.astype(bf),
                dtw_T=dtw.astype(bf),
                dt_bias=np.ascontiguousarray(dtb),
                scales=sc,
                wi8=wi8_np.astype(bf),
                wbc=wbc_np.astype(bf),
                wr12=wr_np.astype(bf),
                ds_diag=dsd.astype(bf),
                wout_T=np.ascontiguousarray(out_w[:, dh].T).astype(bf),
            )
        )
    return in_maps


def kernel(**inputs):
    if "nc" not in _COMPILED:
        _COMPILED["nc"] = _build_nc()
    nc = _COMPILED["nc"]
    in_maps = _prep_in_maps(inputs)
    res = run_bass_kernel_spmd(nc, in_maps, core_ids=list(range(8)))
    out = np.zeros((B, H, W, DM), np.float32)
    for b in range(B):
        p = res.results[2 * b]["out_part"] + res.results[2 * b + 1]["out_part"]
        out[b] = p.T.reshape(H, W, DM)
    return out


# revision 10
# speedup vs baseline: 1.0088x; 1.0041x over previous
"""DSSM (vision Mamba block) Trainium2 kernel — v4.

Problem: B=4, H=W=48, L=2304, D_MODEL=96, D_INNER=192, N=16, R=6, K=3.

Sharding (8 cores, no device-to-device comms):
  core c -> batch b=c//2, d_inner half=c%2. Each core runs the full-d
  front-end (in_proj, depthwise conv, x_dbl) for its batch, the K=3
  selective scans for its 96 d_inner channels, and a partial out_proj
  (contraction over its d-half). Host sums the two partials per batch.

Design notes (measured on HW):
  - DVE scan = ~2.0 ns/col and is the irreducible bottleneck
    (36 groups x [128,2304]); everything else hides under it.
  - x is transposed to [96, L] bf16 on the HOST (the row-major on-device
    load cost ~115us of per-row DMA descriptors).
  - Delta / Delta*u are partition-replicated [96 -> 128=(d8,n16)] by PE
    0/1 matmuls; ACT consumes PSUM directly: a = exp(A*Delta) with a
    per-partition scale vector (exact A), dub copied to SBUF bf16 so the
    w-mul keeps the 2x bf16 DVE rate.
  - all matmuls bf16 (fp32 emits 2 half-speed instructions).
  - LDWEIGHTS elided (inst.ldweights=False) wherever consecutive matmuls
    share a stationary tensor; loops are ordered weight-outer for this.
  - front-end shares the 5 psY PSUM banks as its pipeline ring; psR
    (bufs=3) serves the per-direction streams. 5 + 3 = 8 banks.
  - GpSimd is avoided for bulk work: it contends with DVE SBUF ports
    (+75% measured on concurrent DVE ops).
"""

import numpy as np
import ml_dtypes

import concourse.bass as bass
import concourse.mybir as mybir
import concourse.tile as tile
from concourse.bass_utils import run_bass_kernel_spmd

# ---------------------------------------------------------------- tile fix
# The walrus here accepts only ONE inline sem-wait per instruction; Tile can
# attach several. Hoist extras onto same-engine NOPs placed just before.
_wsplit_counter = [0]


def _split_multi_waits(nc):
    for fn in nc.m.functions:
        for blk in fn.blocks:
            out = []
            changed = False
            for inst in blk.instructions:
                si = inst.sync_info
                waits = list(si.on_wait) if si is not None and si.on_wait else []
                if len(waits) > 1:
                    changed = True
                    for w in waits[:-1]:
                        _wsplit_counter[0] += 1
                        nop = mybir.InstNoOp(name=f"wsplit-{_wsplit_counter[0]}")
                        nop.engine = inst.engine
                        nop.sync_info = mybir.SyncInfo(on_wait=[w], on_update=[])
                        out.append(nop)
                    inst.sync_info = mybir.SyncInfo(
                        on_wait=[waits[-1]],
                        on_update=list(si.on_update) if si.on_update else [],
                    )
                out.append(inst)
            if changed:
                blk.instructions = out


class TileContextFixed(tile.TileContext):
    def __exit__(self, exc_type, exc_val, exc_tb):
        r = super().__exit__(exc_type, exc_val, exc_tb)
        if exc_type is None:
            _split_multi_waits(self.nc)
        return r


# ---------------------------------------------------------------- constants
B, H, W = 4, 48, 48
DM, DI, N, R, K = 96, 192, 16, 6, 3
L = H * W
DH = 96          # d-half per core
G = DH // 8      # 12 groups of 8 channels
TILES = [(0, 480), (480, 960), (960, 1440), (1440, 1920), (1920, 2304)]

F32 = mybir.dt.float32
BF16 = mybir.dt.bfloat16
MUL = mybir.AluOpType.mult
ADD = mybir.AluOpType.add
AF = mybir.ActivationFunctionType

_COMPILED = {}


def _build_nc():
    nc = bass.Bass()

    # ---- dram I/O (per-core values supplied via in_maps)
    xt_in = nc.dram_tensor("xt_in", [DM, L], BF16, kind="ExternalInput")
    wxz = nc.dram_tensor("wxz", [DM, 288], BF16, kind="ExternalInput")
    conv_diag = nc.dram_tensor("conv_diag", [DH, 18 * DH], BF16, kind="ExternalInput")
    conv_bias = nc.dram_tensor("conv_bias", [DH, 2], F32, kind="ExternalInput")
    xp_T = nc.dram_tensor("xp_T", [DH, K * 2 * 80], BF16, kind="ExternalInput")
    dtw_T = nc.dram_tensor("dtw_T", [R, K * DH], BF16, kind="ExternalInput")
    dt_bias = nc.dram_tensor("dt_bias", [DH, K], F32, kind="ExternalInput")
    scales = nc.dram_tensor("scales", [128, K * G], F32, kind="ExternalInput")
    wi8 = nc.dram_tensor("wi8", [DH, G * 128], BF16, kind="ExternalInput")
    wbc = nc.dram_tensor("wbc", [80, 128], BF16, kind="ExternalInput")
    wr12 = nc.dram_tensor("wr12", [128, G * DH], BF16, kind="ExternalInput")
    ds_diag = nc.dram_tensor("ds_diag", [DH, K * DH], BF16, kind="ExternalInput")
    wout_T = nc.dram_tensor("wout_T", [DH, DM], BF16, kind="ExternalInput")
    out_part = nc.dram_tensor("out_part", [DM, L], F32, kind="ExternalOutput")

    with TileContextFixed(nc) as tc:
        with (
            tc.tile_pool(name="wts", bufs=1) as wts,
            tc.tile_pool(name="big", bufs=1) as big,
            tc.tile_pool(name="perd", bufs=2) as perd,
            tc.tile_pool(name="grp", bufs=2) as grp,
            tc.tile_pool(name="psR", bufs=3, space="PSUM") as psR,
            tc.tile_pool(name="psY", bufs=1, space="PSUM") as psY,
        ):
            # ---- load weights
            def wload(dram, shape, dtype):
                t = wts.tile(shape, dtype, tag=dram.name + "_s", name=dram.name + "_s")
                nc.sync.dma_start(t[:, :], dram[:, :])
                return t

            s_wxz = wload(wxz, [DM, 288], BF16)
            s_cd = wload(conv_diag, [DH, 18 * DH], BF16)
            s_cb = wload(conv_bias, [DH, 2], F32)
            s_xp = wload(xp_T, [DH, K * 2 * 80], BF16)
            s_dtw = wload(dtw_T, [R, K * DH], BF16)
            s_dtb = wload(dt_bias, [DH, K], F32)
            s_sc = wload(scales, [128, K * G], F32)
            s_wi8 = wload(wi8, [DH, G * 128], BF16)
            s_wbc = wload(wbc, [80, 128], BF16)
            s_wr = wload(wr12, [128, G * DH], BF16)
            s_ds = wload(ds_diag, [DH, K * DH], BF16)
            s_wout = wload(wout_T, [DH, DM], BF16)

            # matmul with optional LDWEIGHTS elision (stationary tensor is
            # already resident from the previous matmul)
            def mm(out, w, m, start=None, stop=None, ldw=True):
                inst = nc.tensor.matmul(out, w, m, start=start, stop=stop)
                if not ldw:
                    inst.ldweights = False
                return inst

            # front-end PSUM ring: rotate through the 5 psY banks
            _fe = [0]

            def fe_ps(rows):
                t = psY.tile([rows, 480], F32, tag=f"psY{_fe[0] % 5}",
                             name=f"fe{_fe[0]}")
                _fe[0] += 1
                return t

            # ---- x (host-transposed): [96, L] bf16
            xT = big.tile([DM, L], BF16, tag="xT")
            for h0, h1 in [(0, 1152), (1152, L)]:
                nc.sync.dma_start(xT[:, h0:h1], xt_in[:, h0:h1])

            # ---- pads for conv (one per half), zeroed borders (gpsimd: runs
            # once, before any DVE work it could contend with)
            pads = [big.tile([DH, 50 * 50], BF16, tag=f"pad{h}", name=f"pad{h}")
                    for h in range(2)]
            for p in pads:
                nc.gpsimd.memset(p[:, :], 0.0)

            # ---- in_proj: xc (both halves, into pad layout) + z half.
            # weight-outer so each of the 3 stationary tensors loads once.
            zs = big.tile([DH, L], BF16, tag="zs")
            for hh in range(2):
                for it, (t0, t1) in enumerate(TILES):
                    tw = t1 - t0
                    rows = tw // 48
                    ps = fe_ps(DH)
                    mm(ps[:, :tw], s_wxz[:, 96 * hh : 96 * hh + 96],
                       xT[:, t0:t1], ldw=(it == 0))
                    dst = pads[hh][:, :].rearrange("p (r c) -> p r c", r=50, c=50)[
                        :, 1 + 10 * it : 1 + 10 * it + rows, 1:49
                    ]
                    src = ps[:, :tw].rearrange("p (r c) -> p r c", r=rows, c=48)
                    nc.scalar.copy(dst, src)

            # ---- depthwise conv 3x3 + bias + silu -> u (per half), bf16.
            # row-block sets of 3 held in PSUM, tap-outer: 1 ldweights per
            # (half, set, tap) instead of per matmul.
            us = [big.tile([DH, L], BF16, tag=f"u{h}", name=f"u{h}") for h in range(2)]
            for st in range(2):
                for hh in range(2):
                    rbs = [3 * st, 3 * st + 1, 3 * st + 2]
                    pss = [fe_ps(DH) for _ in rbs]
                    j = 0
                    for dy in range(3):
                        for dx in range(3):
                            for i, rb in enumerate(rbs):
                                src = pads[hh][:, :].rearrange(
                                    "p (r c) -> p r c", r=50, c=50
                                )[:, 8 * rb + dy : 8 * rb + dy + 8, dx : dx + 48]
                                mm(pss[i][:, :384],
                                   s_cd[:, (hh * 9 + j) * DH : (hh * 9 + j + 1) * DH],
                                   src, start=(j == 0), stop=(j == 8),
                                   ldw=(i == 0))
                            j += 1
                    for i, rb in enumerate(rbs):
                        nc.scalar.activation(
                            us[hh][:, rb * 384 : rb * 384 + 384],
                            pss[i][:, :384],
                            AF.Silu,
                            bias=s_cb[:, hh : hh + 1],
                        )

            # ---- x_dbl per direction: [80, L] bf16, sections dt@0 B@32 C@64.
            # half-outer with 5 tiles live: 2 ldweights per direction.
            xdbls = [big.tile([80, L], BF16, tag=f"xdbl{k}", name=f"xdbl{k}")
                     for k in range(K)]

            def emit_xdbl(k):
                pss = [fe_ps(80) for _ in TILES]
                for hh in range(2):
                    for it, (t0, t1) in enumerate(TILES):
                        tw = t1 - t0
                        mm(pss[it][:, :tw],
                           s_xp[:, (2 * k + hh) * 80 : (2 * k + hh + 1) * 80],
                           us[hh][:, t0:t1], start=(hh == 0), stop=(hh == 1),
                           ldw=(it == 0))
                for it, (t0, t1) in enumerate(TILES):
                    nc.scalar.copy(xdbls[k][:, t0:t1], pss[it][:, : t1 - t0])

            def emit_z():
                for it, (t0, t1) in enumerate(TILES):
                    tw = t1 - t0
                    ps = fe_ps(DH)
                    mm(ps[:, :tw], s_wxz[:, 192:288], xT[:, t0:t1],
                       ldw=(it == 0))
                    nc.scalar.activation(zs[:, t0:t1], ps[:, :tw], AF.Silu)

            emit_xdbl(0)

            # ---------------- per-direction prep (emitted in slices)
            def emit_prep_xk(k):
                if k == 0:
                    xk = xdbls[0]
                    u_k = us[0]
                elif k == 1:
                    xk = perd.tile([80, L], BF16, tag="xkp")
                    src = xdbls[1][:, :].rearrange("p (h w) -> p w h", h=H, w=W)
                    nc.scalar.copy(
                        xk[:, :].rearrange("p (a b) -> p a b", a=W, b=H), src)
                    u_k = perd.tile([DH, L], BF16, tag="ukp")
                    src = us[0][:, :].rearrange("p (h w) -> p w h", h=H, w=W)
                    nc.scalar.copy(
                        u_k[:, :].rearrange("p (a b) -> p a b", a=W, b=H), src)
                else:
                    # direction 2 runs the scan backwards over forward data
                    # (reversed APs); no permuted copies needed.
                    xk = xdbls[2]
                    u_k = us[0]
                return dict(xk=xk, u_k=u_k)

            def emit_prep_bc(P, sec):
                # B/C partition-broadcast (n-minor): [128, L] bf16 via PE
                xk = P["xk"]
                dstt = perd.tile([128, L], BF16,
                                 tag=("b_b" if sec == 32 else "c_b"))
                for it, (t0, t1) in enumerate(TILES):
                    tw = t1 - t0
                    psb = psR.tile([128, 480], F32, tag="psR")
                    mm(psb[:, :tw], s_wbc[sec : sec + 16, :],
                       xk[sec : sec + 16, t0:t1], ldw=(it == 0))
                    nc.scalar.copy(dstt[:, t0:t1], psb[:, :tw])
                return dstt

            def emit_prep_delta(k, P):
                # delta = ln(exp(v)+1), v = dtw @ dts + bias; bf16 [96, L]
                xk = P["xk"]
                delta = perd.tile([DH, L], BF16, tag="delta")
                for it, (t0, t1) in enumerate(TILES):
                    tw = t1 - t0
                    psv = psR.tile([DH, 480], F32, tag="psR")
                    mm(psv[:, :tw], s_dtw[:, k * DH : (k + 1) * DH],
                       xk[0:R, t0:t1], ldw=(it == 0))
                    ev = grp.tile([DH, 480], F32, tag="ev")
                    nc.scalar.activation(
                        ev[:, :tw], psv[:, :tw], AF.Exp,
                        bias=s_dtb[:, k : k + 1],
                    )
                    nc.scalar.activation(delta[:, t0:t1], ev[:, :tw], AF.Ln,
                                         bias=1.0)
                return delta

            def emit_prep_du(k, parts):
                P = parts[0]
                b_b, c_b, delta = parts[1], parts[2], parts[3]
                du = perd.tile([DH, L], BF16, tag="du")
                nc.vector.tensor_mul(du[:, :], delta[:, :], P["u_k"][:, :])
                return dict(xk=P["xk"], u_k=P["u_k"], b_b=b_b, c_b=c_b,
                            delta=delta, du=du)

            def emit_prep(k):
                P = emit_prep_xk(k)
                b_b = emit_prep_bc(P, 32)
                c_b = emit_prep_bc(P, 64)
                delta = emit_prep_delta(k, P)
                return emit_prep_du(k, [P, b_b, c_b, delta])

            # ---------------- directions
            ysb = [big.tile([DH, L], BF16, tag=f"ysb{k}", name=f"ysb{k}")
                   for k in range(K)]
            prep = emit_prep(0)
            for k in range(K):
                P = prep
                u_k, b_b, c_b, delta, du = (P["u_k"], P["b_b"], P["c_b"],
                                            P["delta"], P["du"])
                psy = [psY.tile([DH, 480], F32, tag=f"psY{c}", name=f"psy{c}_{k}")
                       for c in range(len(TILES))]

                pend = []     # [(g, ch), ...] awaiting y-reduction (depth 2
                              # so PE's repl stream never waits on a fresh ch)
                for g in range(G):
                    # PE replication [96 -> 128=(d8,n16)] via 0/1 matmuls;
                    # wi8_g stays resident for all 10 chunk-matmuls.
                    a_t = grp.tile([128, L], BF16, tag="a")
                    dub = grp.tile([128, L], BF16, tag="dub")
                    first = True
                    for t0, t1 in TILES:
                        tw = t1 - t0
                        psr = psR.tile([128, 480], F32, tag="psR")
                        mm(psr[:, :tw], s_wi8[:, g * 128 : (g + 1) * 128],
                           delta[:, t0:t1], ldw=first)
                        first = False
                        nc.scalar.activation(
                            a_t[:, t0:t1], psr[:, :tw], AF.Exp,
                            scale=s_sc[:, k * G + g : k * G + g + 1],
                        )
                        psd = psR.tile([128, 480], F32, tag="psR")
                        mm(psd[:, :tw], s_wi8[:, g * 128 : (g + 1) * 128],
                           du[:, t0:t1], ldw=False)
                        nc.scalar.copy(dub[:, t0:t1], psd[:, :tw])
                    # w = (delta*u) * B
                    w_t = grp.tile([128, L], BF16, tag="w")
                    nc.vector.tensor_mul(w_t[:, :], dub[:, :], b_b[:, :])
                    # scan along t (2 chained halves; dir 2 scans backwards
                    # through reversed views so h lands in forward order)
                    h_t = grp.tile([128, L], BF16, tag="h")
                    if k < 2:
                        nc.vector.tensor_tensor_scan(
                            h_t[:, 0:1152], a_t[:, 0:1152], w_t[:, 0:1152],
                            0.0, MUL, ADD,
                        )
                        nc.vector.tensor_tensor_scan(
                            h_t[:, 1152:L], a_t[:, 1152:L], w_t[:, 1152:L],
                            h_t[:, 1151:1152], MUL, ADD,
                        )
                    else:
                        nc.vector.tensor_tensor_scan(
                            h_t[:, 1152:L][:, ::-1], a_t[:, 1152:L][:, ::-1],
                            w_t[:, 1152:L][:, ::-1], 0.0, MUL, ADD,
                        )
                        nc.vector.tensor_tensor_scan(
                            h_t[:, 0:1152][:, ::-1], a_t[:, 0:1152][:, ::-1],
                            w_t[:, 0:1152][:, ::-1], h_t[:, 1152:1153],
                            MUL, ADD,
                        )
                    # ch = h * C
                    ch = grp.tile([128, L], BF16, tag="ch", bufs=4)
                    nc.vector.tensor_mul(ch[:, :], h_t[:, :], c_b[:, :])
                    # y-reduce two groups behind so PE never waits on ch
                    if len(pend) >= 2:
                        pg, pch = pend.pop(0)
                        for c, (t0, t1) in enumerate(TILES):
                            mm(psy[c][:, : t1 - t0],
                               s_wr[:, pg * DH : (pg + 1) * DH],
                               pch[:, t0:t1], start=(pg == 0), stop=False,
                               ldw=(c == 0))
                    pend.append((g, ch))
                    # deferred front-end pieces hide inside dir-0's stream
                    if k == 0:
                        if g == 0:
                            emit_xdbl(1)
                        elif g == 2:
                            emit_xdbl(2)
                        elif g == 10:
                            emit_z()
                    # prefetch next direction's prep in slices
                    if k + 1 < K:
                        if g == 3:
                            prep_parts = [emit_prep_xk(k + 1)]
                        elif g == 5:
                            prep_parts.append(emit_prep_bc(prep_parts[0], 32))
                        elif g == 6:
                            prep_parts.append(emit_prep_bc(prep_parts[0], 64))
                        elif g == 8:
                            prep_parts.append(emit_prep_delta(k + 1,
                                                             prep_parts[0]))
                        elif g == 10:
                            prep = emit_prep_du(k + 1, prep_parts)
                # flush the deferred y-reductions
                for pg, pch in pend:
                    for c, (t0, t1) in enumerate(TILES):
                        mm(psy[c][:, : t1 - t0],
                           s_wr[:, pg * DH : (pg + 1) * DH],
                           pch[:, t0:t1], start=(pg == 0), stop=False,
                           ldw=(c == 0))
                # y += Ds * u  (diagonal matmul closes the accumulation)
                for c, (t0, t1) in enumerate(TILES):
                    mm(psy[c][:, : t1 - t0],
                       s_ds[:, k * DH : (k + 1) * DH],
                       u_k[:, t0:t1], start=False, stop=True, ldw=(c == 0))
                for c, (t0, t1) in enumerate(TILES):
                    nc.scalar.copy(ysb[k][:, t0:t1], psy[c][:, : t1 - t0])

            # ---- merge directions (undo orderings), gate, out — chunked
            # so the tail streams instead of serializing full-L passes
            out_sb = big.tile([DM, L], F32, tag="pad0", name="out_sb")
            p1full = ysb[1][:, :].rearrange("p (w h) -> p h w", w=W, h=H)
            y0v = ysb[0][:, :].rearrange("p (a b) -> p a b", a=H, b=W)
            for it, (t0, t1) in enumerate(TILES):
                tw = t1 - t0
                r0, r1 = t0 // W, t1 // W
                nc.vector.tensor_add(y0v[:, r0:r1, :], y0v[:, r0:r1, :],
                                     p1full[:, r0:r1, :])
                nc.vector.tensor_add(ysb[0][:, t0:t1], ysb[0][:, t0:t1],
                                     ysb[2][:, t0:t1])
                nc.vector.tensor_mul(ysb[0][:, t0:t1], ysb[0][:, t0:t1],
                                     zs[:, t0:t1])
                ps = psR.tile([DM, 480], F32, tag="psR")
                mm(ps[:, :tw], s_wout[:, :], ysb[0][:, t0:t1], ldw=(it == 0))
                nc.scalar.copy(out_sb[:, t0:t1], ps[:, :tw])
                nc.sync.dma_start(out_part[:, t0:t1], out_sb[:, t0:t1])

    return nc


def _prep_in_maps(inputs):
    f32 = lambda a: np.ascontiguousarray(np.asarray(a, np.float32))
    x = f32(inputs["x"])
    in_proj_w = f32(inputs["in_proj_w"])        # (384, 96)
    conv_w = f32(inputs["conv_w"]).reshape(DI, 9)
    conv_b = f32(inputs["conv_b"])
    x_proj_w = f32(inputs["x_proj_w"])          # (K, 38, 192)
    dt_w = f32(inputs["dt_projs_w"])            # (K, 192, 6)
    dt_b = f32(inputs["dt_projs_b"])            # (K, 192)
    A = -np.exp(f32(inputs["A_logs"])).reshape(K, DI, N)
    Ds = f32(inputs["Ds"]).reshape(K, DI)
    out_w = f32(inputs["out_proj_w"])           # (96, 192)

    # y-reduction: psy[d'] = sum_p wr[p, d'] * ch[p];  p = d8*16 + n
    wr_np = np.zeros((128, G * DH), np.float32)
    for g in range(G):
        for d8 in range(8):
            wr_np[d8 * 16 : d8 * 16 + 16, g * DH + g * 8 + d8] = 1.0

    # group-select replication: wi8[g*8+d8, g*128 + d8*16 + n] = 1
    wi8_np = np.zeros((DH, G * 128), np.float32)
    for g in range(G):
        for d8 in range(8):
            wi8_np[g * 8 + d8, g * 128 + d8 * 16 : g * 128 + d8 * 16 + 16] = 1.0

    # B/C broadcast: row 32+n (or 64+n) -> partitions d8*16+n
    wbc_np = np.zeros((80, 128), np.float32)
    for n in range(16):
        wbc_np[32 + n, n::16] = 1.0
        wbc_np[64 + n, n::16] = 1.0

    in_maps = []
    for c in range(8):
        b, half = c // 2, c % 2
        pd = np.concatenate([np.arange(DI)[96 * half : 96 * half + 96],
                             np.arange(DI)[96 * (1 - half) : 96 * (1 - half) + 96]])
        dh = pd[:DH]

        wxz_np = np.zeros((DM, 288), np.float32)
        wxz_np[:, 0:96] = in_proj_w[pd[:96]].T
        wxz_np[:, 96:192] = in_proj_w[pd[96:]].T
        wxz_np[:, 192:288] = in_proj_w[DI + dh].T

        cd = np.zeros((DH, 18 * DH), np.float32)
        for hh in range(2):
            ch_idx = pd[hh * 96 : hh * 96 + 96]
            for j in range(9):
                blk = np.zeros((DH, DH), np.float32)
                np.fill_diagonal(blk, conv_w[ch_idx, j])
                cd[:, (hh * 9 + j) * DH : (hh * 9 + j + 1) * DH] = blk
        cb = np.stack([conv_b[pd[:96]], conv_b[pd[96:]]], axis=1)

        xp = np.zeros((DH, K * 2 * 80), np.float32)
        for k in range(K):
            for hh in range(2):
                blk = np.zeros((DH, 80), np.float32)
                ch_idx = pd[hh * 96 : hh * 96 + 96]
                blk[:, 0:6] = x_proj_w[k][0:6, ch_idx].T
                blk[:, 32:48] = x_proj_w[k][6:22, ch_idx].T
                blk[:, 64:80] = x_proj_w[k][22:38, ch_idx].T
                xp[:, (2 * k + hh) * 80 : (2 * k + hh + 1) * 80] = blk

        dtw = np.zeros((R, K * DH), np.float32)
        for k in range(K):
            dtw[:, k * DH : (k + 1) * DH] = dt_w[k][dh].T
        dtb = np.stack([dt_b[k][dh] for k in range(K)], axis=1)

        # per-partition decay scales: scales[p, k*G+g] = A[k, dh[g*8+d8], n]
        # with p = d8*16 + n
        sc = np.zeros((128, K * G), np.float32)
        for k in range(K):
            for g in range(G):
                for d8 in range(8):
                    sc[d8 * 16 : d8 * 16 + 16, k * G + g] = A[k, dh[g * 8 + d8]]

        dsd = np.zeros((DH, K * DH), np.float32)
        for k in range(K):
            np.fill_diagonal(dsd[:, k * DH : (k + 1) * DH], Ds[k][dh])

        bf = ml_dtypes.bfloat16
        in_maps.append(
            dict(
                xt_in=np.ascontiguousarray(x[b].reshape(L, DM).T).astype(bf),
                wxz=wxz_np.astype(bf),
                conv_diag=cd.astype(bf),
                conv_bias=np.ascontiguousarray(cb),
                xp_T=xp.astype(bf),
                dtw_T=dtw.astype(bf),
                dt_bias=np.ascontiguousarray(dtb),
                scales=sc,
                wi8=wi8_np.astype(bf),
                wbc=wbc_np.astype(bf),
                wr12=wr_np.astype(bf),
                ds_diag=dsd.astype(bf),
                wout_T=np.ascontiguousarray(out_w[:, dh].T).astype(bf),
            )
        )
    return in_maps


def kernel(**inputs):
    if "nc" not in _COMPILED:
        _COMPILED["nc"] = _build_nc()
    nc = _COMPILED["nc"]
    in_maps = _prep_in_maps(inputs)
    res = run_bass_kernel_spmd(nc, in_maps, core_ids=list(range(8)))
    out = np.zeros((B, H, W, DM), np.float32)
    for b in range(B):
        p = res.results[2 * b]["out_part"] + res.results[2 * b + 1]["out_part"]
        out[b] = p.T.reshape(H, W, DM)
    return out
